# revision 34
# baseline (speedup 1.0000x reference)
"""Blake2 soft-cipher Bass kernel for Trainium2 (8 NeuronCores, data parallel)."""
import sys
sys.path.insert(0, "/opt/trn_rl_repo")
import math
import os
import numpy as np
from concourse import bass, mybir
from concourse.tile import TileContext
from concourse.bass_primitives_rust import SemaphoreHandle
from concourse import bass_primitives_rust as _bpr
from concourse.bass import _bass_rust

A = mybir.AluOpType
F = mybir.ActivationFunctionType
DT = mybir.dt.float32

# ---------------------------------------------------------------- geometry
P = 128
FD = 490
BLOCK_ROWS = P * FD
BLOCKS = 4
CORE_ROWS = BLOCK_ROWS * BLOCKS
N_CORES = 8
TOTAL_ROWS = 2_000_000
PAD_ROWS = CORE_ROWS * N_CORES

ROUNDS = 10
G_SCHEDULE = [
    (0, 4, 8, 12, 0, 1), (1, 5, 9, 13, 2, 3), (2, 6, 10, 14, 4, 5), (3, 7, 11, 15, 6, 7),
    (0, 5, 10, 15, 8, 9), (1, 6, 11, 12, 10, 11), (2, 7, 8, 13, 12, 13), (3, 4, 9, 14, 14, 15),
]
_IV_INTS = [7640891576956012808, 13503953896175478587, 4354685564936845355,
            11912009170470909681, 5840696475078001361, 11170449401992604703,
            2270897969802886507, 6620516959819538809]
IV = (np.asarray(_IV_INTS, dtype=np.float32) / np.float32(2.0**64)).astype(np.float32)
STEEP = np.float32(10.0)

# A pending scale at or below 2^-14 pins the soft_xor sigmoid argument
# 10*(v*scale - 0.5) to [-5, -5+1.6e-4]: the sigmoid deviates from
# sigmoid(-5) by <= 1e-6, below the activation table's own noise, so it is
# treated as a compile-time constant.  (At scale <= 2^-26 this is exact:
# the argument rounds to -5.0 in f32.)
TINY_SCALE = 2.0 ** -14
SCR_BUFS = 3
GPS_BIAS = 0.95
STAGGER = 0
N_STREAMS = 2
PAIR_SIG = True
PAIR_STT = False
PAIR_TT = False
PAIR_TS = False


def f32(x):
    return np.float32(x)


def sig_const(z):
    return f32(1.0 / (1.0 + math.exp(-float(z))))


def configure(fd=980, blocks=2, n_streams=None):
    global FD, BLOCK_ROWS, BLOCKS, CORE_ROWS, PAD_ROWS, N_STREAMS
    FD = fd
    BLOCKS = blocks
    if n_streams is not None:
        N_STREAMS = n_streams
    BLOCK_ROWS = P * FD
    CORE_ROWS = BLOCK_ROWS * BLOCKS
    PAD_ROWS = CORE_ROWS * N_CORES


class Val:
    def __init__(self, const=None, ap=None, scale=None):
        self.const = const
        self.ap = ap          # () -> AP
        self.scale = scale    # pending multiply-by-2^-n (rotate folding)

    @property
    def is_const(self):
        return self.const is not None


# per-op cost model (ns), fitted to TimelineSim at FD in {512, 980, 1960}
def ns_dve_tt(w=1):
    return FD * w * 1.0414 + 155.0

def ns_dve_ts(w=1):
    return FD * w * 0.5207 + 153.0

def ns_gps_tt(w=1):
    return FD * w * 1.9841 + 156.0

def ns_act(w=1):
    return FD * w * 0.8329 + 405.0


class Program2:
    def __init__(self):
        self.nc = bass.Bass("TRN2")
        self.est = {"dve": 0.0, "gps": 0.0, "act": 0.0}
        self._lane = None
        self._lane_id = 0
        self._pend = {}
        self._scr_pend = {}

    def _run(self, fn):
        fn()

    def begin_lane(self, lane, lane_id=0):
        self._lane = lane
        self._lane_id = lane_id

    def end_lane(self):
        self._lane = None

    def merge_lanes(self, lanes):
        # Stagger lane start offsets so the lanes sit at different positions
        # of the add->sigmoid->sub chain: without this all lanes issue the
        # same op type at once and the engines take turns idling.
        lanes = [list(l) for l in lanes if l]
        for i, l in enumerate(lanes):
            for _ in range(min(STAGGER * i, len(l))):
                l.pop(0)()
        lanes = [l for l in lanes if l]
        while lanes:
            nxt = []
            for l in lanes:
                l.pop(0)()
                if l:
                    nxt.append(l)
            lanes = nxt

    def _li(self):
        return (self._lane_id or 0) % 4

    def _stream(self):
        return (self._lane_id or 0) // 4

    def _pair(self, cls, enabled, aps, scalars):
        """Cross-stream op pairing: stream 0 defers, stream 1 emits one
        double-width op over the paired APs.  Returns None (emit narrow),
        or False (deferred), or the list of pair-ap getters (emit wide)."""
        if True or N_STREAMS != 2 or not enabled:
            return None
        if any(not hasattr(a, "pair") for a in aps):
            return None
        key = (cls, self._li())
        if self._stream() == 0:
            self._pend.setdefault(key, []).append(scalars)
            return False
        rec = self._pend[key].pop(0)
        assert rec == scalars, (cls, rec, scalars)
        return [a.pair for a in aps]

    # ---------- low-level emitters (inside TileContext)
    def dve_tt(self, out, a, b, op, gps_ok=False):
        """Plain two-tensor op; may run on GPSIMD (Pool) when gps_ok."""
        pr = self._pair("tt", PAIR_TT, [out, a, b], (op,))
        if pr is False:
            self._run(lambda: None)
            return
        w = 1
        if pr is not None:
            out, a, b = pr
            w = 2
        use_gps = gps_ok and (self.est["gps"] + ns_gps_tt(w) * GPS_BIAS <
                              self.est["dve"] + ns_dve_tt(w))
        if use_gps:
            self._run(lambda: self.nc.gpsimd.tensor_tensor(out(), a(), b(), op=op))
            self.est["gps"] += ns_gps_tt(w)
        else:
            self._run(lambda: self.nc.vector.tensor_tensor(out(), a(), b(), op=op))
            self.est["dve"] += ns_dve_tt(w)

    def dve_stt(self, out, in0, scalar, in1, op0, op1, rev0=False, rev1=False):
        pr = self._pair("stt", PAIR_STT, [out, in0, in1], (scalar, op0, op1, rev0, rev1))
        if pr is False:
            self._run(lambda: None)
            return
        w = 1
        if pr is not None:
            out, in0, in1 = pr
            w = 2
        def f():
            i = self.nc.vector.scalar_tensor_tensor(out(), in0(), scalar, in1(), op0=op0, op1=op1)
            if rev0:
                i.ins.reverse0 = True
            if rev1:
                i.ins.reverse1 = True
        self._run(f)
        self.est["dve"] += ns_dve_tt(w)

    def dve_ts(self, out, in0, s1, s2, op0, op1=None, rev0=False):
        pr = self._pair("ts", PAIR_TS, [out, in0], (s1, s2, op0, op1, rev0))
        if pr is False:
            self._run(lambda: None)
            return
        w = 1
        if pr is not None:
            out, in0 = pr
            w = 2
        def f():
            if op1 is None:
                i = self.nc.vector.tensor_scalar(out(), in0(), s1, None, op0=op0)
            else:
                i = self.nc.vector.tensor_scalar(out(), in0(), s1, s2, op0=op0, op1=op1)
            if rev0:
                i.ins.reverse0 = True
        self._run(f)
        self.est["dve"] += ns_dve_ts(w)

    def act_act(self, out, in0, func, bias, scale, bias_ap=None):
        pr = self._pair("sig", PAIR_SIG, [out, in0], (func, bias, scale, id(bias_ap)))
        if pr is False:
            self._run(lambda: None)
            return
        w = 1
        if pr is not None:
            out, in0 = pr
            w = 2
        def f():
            b = bias_ap() if bias_ap is not None else bias
            self.nc.scalar.activation(out(), in0(), func, bias=b, scale=scale)
        self._run(f)
        self.est["act"] += ns_act(w)

    def affine(self, out, in0, scale, bias):
        if self.est["act"] + ns_act() < self.est["dve"] + ns_dve_ts():
            self.act_act(out, in0, F.Copy, float(bias), float(scale))
        else:
            if bias == 0.0:
                self.dve_ts(out, in0, float(scale), None, A.mult)
            else:
                self.dve_ts(out, in0, float(scale), float(bias), A.mult, A.add)

    def affine1m(self, out, in0):
        if self.est["act"] + ns_act() < self.est["dve"] + ns_dve_ts():
            self.act_act(out, in0, F.Copy, 1.0, -1.0)
        else:
            self.dve_ts(out, in0, 1.0, None, A.subtract, rev0=True)

    # ---------- scratch (optionally paired across streams)
    def scr(self, paired=False):
        if False:
            li = self._li()
            if self._stream() == 0:
                t = self.scr_pool.tile([P, 2 * FD], DT, tag=f"ps{li}",
                                       name=f"ps{li}", bufs=SCR_BUFS)
                self._scr_pend.setdefault(li, []).append(t)
            else:
                t = self._scr_pend[li].pop(0)
            s = self._stream()
            g = lambda: t[:][:, s * FD:(s + 1) * FD]
            g.pair = lambda: t[:]
            return g
        cell = {}
        tag = f"scr{self._lane_id or 0}"
        def get():
            if "t" not in cell:
                cell["t"] = self.scr_pool.tile([P, FD], DT, tag=tag, name=tag, bufs=SCR_BUFS)
            return cell["t"][:]
        return get

    # ---------- math primitives
    def sig_or_lin(self, out, val):
        # sigmoid whose argument is pinned to [-5, -5+1.6e-4] by a deferred
        # rotate scale: the linearization error is < 0.1 ulp of the output,
        # so a DVE mult-add replaces the ACT sigmoid exactly.
        sc = val.scale
        if sc is not None and float(sc) <= 2.0 ** -14:
            s0 = sig_const(-5.0)
            kappa = f32(10.0) * f32(sc) * f32(f32(s0) * f32(1.0 - float(s0)))
            self.dve_ts(out, val.ap, float(kappa), float(s0), A.mult, A.add)
        else:
            self.sigmoid_act(out, val.ap, "xor", in_scale=sc)

    def sigmoid_act(self, out, in_ap, which, in_scale=None):
        bb = self.bias_m10_ap if which == "add" else self.bias_m5_ap
        sc = float(STEEP) * float(in_scale) if in_scale is not None else float(STEEP)
        self.act_act(out, in_ap, F.Sigmoid, None, sc, bias_ap=bb)

    def soft_add(self, dst_slot, aval, bval, dst_ap=None):
        if aval.is_const and bval.is_const:
            s = f32(aval.const + bval.const)
            wrap = sig_const(STEEP * (s - f32(1.0)))
            return Val(const=f32(s - wrap))
        # A rot32-deferred operand is < 2^-32: adding it to the other word
        # rounds away entirely (exactly, for any other-side value >= 2^-8;
        # the residual deviation elsewhere is ~2^-31, far below tolerance).
        for sv, ov in ((aval, bval), (bval, aval)):
            if (not sv.is_const and sv.ap is not None and sv.scale is not None
                    and float(sv.scale) <= 2.0 ** -31):
                if ov.is_const:
                    s = f32(ov.const)
                    wrap = sig_const(STEEP * (s - f32(1.0)))
                    return Val(const=f32(s - wrap))
                assert ov.scale is None
                dst = dst_ap if dst_ap is not None else self.v_aps[dst_slot]
                w = self.scr(paired=PAIR_SIG)
                self.sigmoid_act(w, ov.ap, "add")
                self.dve_tt(dst, ov.ap, w, A.subtract, gps_ok=True)
                return Val(ap=dst)
        dst = dst_ap if dst_ap is not None else self.v_aps[dst_slot]
        if aval.is_const or bval.is_const:
            c = aval.const if aval.is_const else bval.const
            tv = bval if aval.is_const else aval
            if tv.scale is not None:
                self.dve_ts(dst, tv.ap, float(tv.scale), float(c), A.mult, A.add)
            else:
                self.dve_ts(dst, tv.ap, float(c), None, A.add)
        elif aval.scale is not None or bval.scale is not None:
            sv, ov = (aval, bval) if aval.scale is not None else (bval, aval)
            assert ov.scale is None
            self.dve_stt(dst, sv.ap, float(sv.scale), ov.ap, op0=A.mult, op1=A.add)
        else:
            self.dve_tt(dst, aval.ap, bval.ap, A.add, gps_ok=True)
        w = self.scr(paired=PAIR_SIG)
        self.sigmoid_act(w, dst, "add")
        self.dve_tt(dst, dst, w, A.subtract, gps_ok=True)
        return Val(ap=dst)

    def soft_xor_const(self, b, tval, dst, tsc=None):
        """xor with one side's sigmoid a compile-time constant b.
        r = gamma*ys^2 + beta*ys + alpha, ys = sigmoid(10*(t*tsc) - 5)."""
        alpha = f32(b)
        beta = f32(f32(1.0) - f32(3.0) * b + f32(b * b))
        gamma = f32(b - f32(b * b))
        ys = self.scr(paired=PAIR_SIG); h = self.scr()
        self.sigmoid_act(ys, tval.ap, "xor", in_scale=tsc)
        self.affine(h, ys, float(gamma), float(beta))
        self.dve_tt(dst, h, ys, A.mult, gps_ok=True)
        self.affine(dst, dst, 1.0, float(alpha))
        return Val(ap=dst)

    def soft_xor(self, dst_slot, xval, yval, dst_ap=None):
        if xval.is_const and yval.is_const:
            xs = sig_const(STEEP * (xval.const - f32(0.5)))
            ys = sig_const(STEEP * (yval.const - f32(0.5)))
            t1 = f32(xs * f32(1.0 - ys)); t2 = f32(f32(1.0 - xs) * ys)
            r = f32(f32(t1 + t2) - f32(t1 * t2))
            return Val(const=f32(min(max(r, 0.0), 1.0)))
        dst = dst_ap if dst_ap is not None else self.v_aps[dst_slot]
        # one side has a constant sigmoid: true const, or scaled below 2^-26
        # (the rot32-deferral case: sigmoid arg rounds to exactly -5.0)
        for sv, tv in ((xval, yval), (yval, xval)):
            if sv.is_const:
                b = sig_const(STEEP * (sv.const - f32(0.5)))
                return self.soft_xor_const(b, tv, dst, tsc=tv.scale)
            if (not sv.is_const and sv.ap is not None and sv.scale is not None
                    and float(sv.scale) <= TINY_SCALE):
                b = sig_const(-5.0)
                return self.soft_xor_const(b, tv, dst, tsc=tv.scale)
        # r = t1 + (1-t1)*t2 = t1 + t2 - t1*t2; final add is GPS-eligible
        xs = self.scr(paired=PAIR_SIG); ys = self.scr(paired=PAIR_SIG); t1 = self.scr()
        self.sigmoid_act(xs, xval.ap, "xor", in_scale=xval.scale)
        self.sigmoid_act(ys, yval.ap, "xor", in_scale=yval.scale)
        self.dve_stt(t1, ys, 1.0, xs, op0=A.subtract, op1=A.mult, rev0=True)
        self.dve_stt(xs, xs, 1.0, ys, op0=A.subtract, op1=A.mult, rev0=True)
        self.dve_stt(xs, t1, 1.0, xs, op0=A.subtract, op1=A.mult, rev0=True)
        self.dve_tt(dst, t1, xs, A.add, gps_ok=True)
        return Val(ap=dst)

    def rotate(self, slot, n, val):
        if val.is_const:
            c = f32(val.const)
            if n in (16, 24, 32):
                return Val(const=f32(c * f32(2.0 ** (-n))))
            g = f32(1.0) if float(c) >= 0.5 else f32(0.0)
            return Val(const=f32(f32(2.0) * c - g))
        assert val.scale is None
        if n in (16, 24, 32):
            # soft_xor output >= ~0.0132 > 2^(23-(64-n)) for n<=32: the wrapped
            # fraction is identically zero, so the rotate is an exact scale by
            # 2^-n.  Defer it into the consumers (soft_add STT / sigmoid scale).
            return Val(ap=val.ap, scale=f32(2.0 ** (-n)))
        if n == 63:
            # x in [0,1): frac(2x) = 2x - [x>=0.5].  The reference adds x*2^-63,
            # but that term is < 2^-63: when frac >= 2^-38 it rounds away, and
            # when frac < 2^-38 every downstream consumer (soft_add against an
            # O(1) word, sigmoid with arg ~ -5) rounds identically with or
            # without it.  Drop it.
            x = val.ap
            dst = self.v_aps[slot]
            mask = self.scr()
            self.dve_ts(mask, x, 0.5, None, A.is_ge)
            self.dve_stt(dst, x, 2.0, mask, op0=A.mult, op1=A.subtract)
            return Val(ap=dst)
        raise AssertionError(f"unexpected rotate {n}")

    def G(self, vals, a, b, c, d, xi, yi):
        mx = Val(ap=self.m_aps[xi])
        my = Val(ap=self.m_aps[yi])
        vals[a] = self.soft_add(a, vals[a], vals[b])
        vals[a] = self.soft_add(a, vals[a], mx)
        # v[d] = rot32(soft_xor(v[d], v[a])) is dead code: the deferred 2^-32
        # scale makes the following soft_add absorb to a no-op and the next
        # soft_xor's sigmoid of d a compile-time constant, so no consumer
        # ever reads the value.  Leave a tiny-scaled sentinel.
        vals[d] = Val(ap=self.v_aps[d], scale=f32(2.0 ** -32))
        vals[c] = self.soft_add(c, vals[c], vals[d])
        vals[b] = self.soft_xor(b, vals[b], vals[c])
        vals[b] = self.rotate(b, 24, vals[b])
        vals[a] = self.soft_add(a, vals[a], vals[b])
        vals[a] = self.soft_add(a, vals[a], my)
        vals[d] = self.soft_xor(d, vals[d], vals[a])
        vals[d] = self.rotate(d, 16, vals[d])
        vals[c] = self.soft_add(c, vals[c], vals[d])
        vals[b] = self.soft_xor(b, vals[b], vals[c])
        vals[b] = self.rotate(b, 63, vals[b])

    # ---------- whole program
    def build(self, scr_bufs=12):
        nc = self.nc
        ns = N_STREAMS
        assert BLOCKS % ns == 0
        self.msg = nc.declare_dram_parameter("message", [CORE_ROWS, 16], DT, isOutput=False)
        self.out = nc.declare_dram_parameter("out", [CORE_ROWS, 8], DT, isOutput=True)
        def with_pair(half_fn, pair_fn):
            half_fn.pair = pair_fn
            return half_fn

        with TileContext(nc) as tc:
            with (
                tc.tile_pool(name="persist", bufs=1) as pp,
                tc.tile_pool(name="scrp", bufs=scr_bufs) as sp,
            ):
                self.scr_pool = sp
                m_tiles = [pp.tile([P, 16 * FD], DT, tag=f"m_stage{s}", name=f"m_stage{s}")
                           for s in range(ns)]
                out_tiles = [pp.tile([P, 8 * FD], DT, tag=f"out_stage{s}", name=f"out_stage{s}")
                             for s in range(ns)]
                v_tiles = [[pp.tile([P, FD], DT, tag=f"v{s}_{j}", name=f"v{s}_{j}")
                            for j in range(16)] for s in range(ns)]
                bias_m10 = pp.tile([P, 1], DT, tag="bias_m10", name="bias_m10")
                bias_m5 = pp.tile([P, 1], DT, tag="bias_m5", name="bias_m5")
                nc.vector.memset(bias_m10[:], -10.0)
                nc.vector.memset(bias_m5[:], -5.0)
                self.bias_m10_ap = with_pair(lambda: bias_m10[:], lambda: bias_m10[:])
                self.bias_m5_ap = with_pair(lambda: bias_m5[:], lambda: bias_m5[:])
                v_aps_all = [[(lambda ss=s, jj=j: v_tiles[ss][jj][:]) for j in range(16)]
                             for s in range(ns)]
                m_aps_all = [[(lambda ss=s, jj=j: m_tiles[ss][:][:, jj::16]) for j in range(16)]
                             for s in range(ns)]

                for pair in range(BLOCKS // ns):
                    blks = [pair * ns + s for s in range(ns)]
                    for s, blk in enumerate(blks):
                        r0 = blk * BLOCK_ROWS
                        in_ap = self.msg[r0:r0 + BLOCK_ROWS, :].rearrange(
                            "(p f) w -> p (f w)", p=P)
                        nc.sync.dma_start(out=m_tiles[s][:], in_=in_ap)
                    states = [[Val(const=IV[j]) for j in range(8)] for _ in range(ns)]
                    for rnd in range(ROUNDS):
                        valss = []
                        for s in range(ns):
                            vals = {}
                            for j in range(8):
                                vals[j] = states[s][j]
                                vals[8 + j] = Val(const=IV[j])
                            valss.append(vals)
                        for grp in (G_SCHEDULE[:4], G_SCHEDULE[4:]):
                            lanes = [[] for _ in range(ns * len(grp))]
                            for s in range(ns):
                                self.v_aps = v_aps_all[s]
                                self.m_aps = m_aps_all[s]
                                for li, (a, b, c, d, xi, yi) in enumerate(grp):
                                    self.begin_lane(lanes[s * len(grp) + li], s * 4 + li)
                                    self.G(valss[s], a, b, c, d, xi, yi)
                                    self.end_lane()
                            self.merge_lanes(lanes)
                        last = rnd == ROUNDS - 1
                        lanes = [[] for _ in range(ns * 8)]
                        new_states = [[] for _ in range(ns)]
                        for s in range(ns):
                            self.v_aps = v_aps_all[s]
                            self.m_aps = m_aps_all[s]
                            for j in range(8):
                                self.begin_lane(lanes[s * 8 + j], s * 4 + (j % 4))
                                if last:
                                    dst = (lambda ss=s, jj=j: out_tiles[ss][:][:, jj::8])
                                    new_states[s].append(self.soft_xor(
                                        None, valss[s][j], valss[s][8 + j], dst_ap=dst))
                                else:
                                    new_states[s].append(self.soft_xor(
                                        j, valss[s][j], valss[s][8 + j]))
                                self.end_lane()
                        self.merge_lanes(lanes)
                        states = new_states
                    for s, blk in enumerate(blks):
                        r0 = blk * BLOCK_ROWS
                        out_ap = self.out[r0:r0 + BLOCK_ROWS, :].rearrange(
                            "(p f) w -> p (f w)", p=P)
                        nc.sync.dma_start(out=out_ap, in_=out_tiles[s][:])
        assert not any(self._pend.values()), "unmatched paired ops"
        hoist_excess_waits(nc)
        return nc


def hoist_excess_waits(nc, max_waits=1):
    """Walrus can't encode >~2 sync waits per instruction; move excess into
    standalone NoOps (1 wait each) right before the instruction."""
    n_hoisted = 0
    for f in nc.m.functions:
        for blk in f.blocks:
            need = False
            for inst in blk.instructions:
                si = inst.sync_info
                if si is not None and len(si.on_wait) > max_waits:
                    need = True
                    break
            if not need:
                continue
            newl = []
            for inst in blk.instructions:
                si = inst.sync_info
                if si is not None and len(si.on_wait) > max_waits:
                    conds = list(si.on_wait)
                    keep = conds[-max_waits:]
                    for c in conds[:-max_waits]:
                        nop = mybir.InstNoOp(
                            name=nc.get_next_instruction_name(), ins=[], outs=[])
                        nop.engine = inst.engine
                        _bass_rust.wait_op(
                            nop, SemaphoreHandle(c.ant_name, c.id),
                            c.wait_value, "sem-ge", False)
                        newl.append(nop)
                        n_hoisted += 1
                    inst.sync_info = mybir.SyncInfo(on_wait=keep, on_update=list(si.on_update))
                newl.append(inst)
            blk.instructions = newl
    return n_hoisted


def build_program():
    p = Program2()
    nc = p.build()
    return nc, p


# ----------------------------------------------------------------- entry
_cache = {}


def _get_nc():
    if "nc" not in _cache:
        _cache["nc"] = build_program()[0]
    return _cache["nc"]


def kernel(message, _trace=False):
    """Full (2000000, 16) f32 in -> (2000000, 8) f32 out, 8-core data parallel."""
    from concourse.bass_utils import run_bass_kernel_spmd
    msg = np.ascontiguousarray(np.asarray(message, dtype=np.float32))
    nc = _get_nc()
    pad = PAD_ROWS - msg.shape[0]
    msgp = np.concatenate([msg, np.zeros((pad, 16), np.float32)]) if pad > 0 else msg
    shards = msgp.reshape(N_CORES, CORE_ROWS, 16)
    in_maps = [{"message": shards[i]} for i in range(N_CORES)]
    kw = dict(trace=True) if _trace else {}
    res = run_bass_kernel_spmd(nc, in_maps, core_ids=list(range(N_CORES)), **kw)
    out = np.concatenate([res.results[i]["out"] for i in range(N_CORES)], axis=0)
    if _trace:
        _cache["last_result"] = res
    return out[: msg.shape[0]]
